# revision 10
# baseline (speedup 1.0000x reference)
"""KMeans summarize kernel for Trainium2, data-parallel over N on 8 NeuronCores.

v3: all-bf16 matmuls (FWL-fast weight loads), Relu one-hot in {0,1}.

Per core (62592 rows = 489 tiles of 128):
  PE:   q = CC' - 2*X@C^T via bf16 matmul (stationary [X^T;1;1], moving
        [-2C^T; CChi; CClo], fp32 PSUM accumulate); scatter
        acc[j,k] += sum_i M[i,j]*A[i,k] with M=[X*W | W] bf16, A one-hot
  DVE:  m = min_k q per row; b' = m/delta + 1 (quad-batched)
  ACT:  A = Relu(-q/delta + b')  -- exactly 1 at the argmin, 0 elsewhere
        (Sterbenz: at argmin q==m so -m/d + m/d + 1 == 1 exactly)
Host: XW=X*W and XX=sum X^2 precomputed; CC row centered and split into
bf16 hi/lo pair. inertia = sum sqrt(clip(XX+m+cc0,0)/64) on device.
"""

import sys

sys.path.insert(0, "/opt/trn_rl_repo")

import numpy as np
import ml_dtypes

N, D, K = 500_000, 64, 512
NCORES = 8
P = 128
TILES = 489                      # per-core tiles
ROWS = P * TILES                 # 62592 per core
NPAD = NCORES * ROWS             # 500736
GROUP = 8
DELTA = 2.0 ** -12
_GROUPS = [GROUP] * (TILES // GROUP) + ([TILES % GROUP] if TILES % GROUP else [])

_CACHE = {}


def _row_index(tiles=TILES, groups=_GROUPS):
    """Original row index for (tile t, partition p) ordering used on device."""
    idx = np.empty(tiles * P, dtype=np.int64)
    base = 0
    t = 0
    for g in groups:
        for j in range(g):
            tt = t + j
            idx[tt * P:(tt + 1) * P] = base + np.arange(P) * g + j
        base += g * P
        t += g
    return idx


def _build(groups=None):
    import concourse.bass as bass
    import concourse.mybir as mybir
    import concourse.tile as tile

    if groups is None:
        groups = _GROUPS
    tiles = sum(groups)

    fp32 = mybir.dt.float32
    bf16 = mybir.dt.bfloat16
    nc = bass.Bass()
    wf = nc.dram_tensor("wf", (ROWS, 2 * D), bf16, kind="ExternalInput")
    xt = nc.dram_tensor("xt", (D + 2, ROWS), bf16, kind="ExternalInput")
    rhs = nc.dram_tensor("rhs", (D + 2, K), bf16, kind="ExternalInput")
    xx = nc.dram_tensor("xx", (P, tiles), fp32, kind="ExternalInput")
    cc0 = nc.dram_tensor("cc0", (P, 1), fp32, kind="ExternalInput")
    out = nc.dram_tensor("out", (P, K + 4), fp32, kind="ExternalOutput")

    AX = mybir.AxisListType.X
    OP = mybir.AluOpType
    AF = mybir.ActivationFunctionType

    from contextlib import ExitStack

    with tile.TileContext(nc) as tc, ExitStack() as es:
        consts = es.enter_context(tc.tile_pool(name="consts", bufs=1))
        xtp = es.enter_context(tc.tile_pool(name="xtp", bufs=3))
        wfp = es.enter_context(tc.tile_pool(name="wfp", bufs=3))
        ap_ = es.enter_context(tc.tile_pool(name="ap_", bufs=7))
        dpp = es.enter_context(tc.tile_pool(name="dpp", bufs=5, space="PSUM"))
        app = es.enter_context(tc.tile_pool(name="app", bufs=1, space="PSUM"))

        rhs_sb = consts.tile([D + 2, K], bf16)
        nc.sync.dma_start(out=rhs_sb, in_=rhs.ap())
        xxbuf = consts.tile([P, tiles], fp32)
        nc.sync.dma_start(out=xxbuf, in_=xx.ap())
        cc0_sb = consts.tile([P, 1], fp32)
        nc.sync.dma_start(out=cc0_sb, in_=cc0.ap())
        mbuf = consts.tile([P, tiles], fp32)
        bbuf = consts.tile([P, tiles], fp32)
        scat = app.tile([P, K], fp32)            # scatter accumulator (1 bank)

        pend = []          # (a_tile, wfg, j, t) awaiting scatter matmul
        sgn = []           # (dist, wfg, j, t) awaiting bias + relu
        t = 0
        base_row = 0
        for g in groups:
            # ---- group loads ----
            xtg = xtp.tile([D + 2, GROUP * P], bf16, tag="xtg")
            nc.sync.dma_start(
                out=xtg[:, : g * P], in_=xt.ap()[:, t * P:(t + g) * P]
            )
            wfg = wfp.tile([P, GROUP, 2 * D], bf16, tag="wfg")
            src_w = bass.AP(
                tensor=wf, offset=base_row * 2 * D,
                ap=[[g * 2 * D, P], [2 * D, g], [1, 2 * D]],
            )
            nc.sync.dma_start(out=wfg[:, :g, :], in_=src_w)

            for j in range(g):
                dist = dpp.tile([P, K], fp32, tag="dist")
                nc.tensor.matmul(
                    dist, xtg[:, j * P:(j + 1) * P], rhs_sb,
                    start=True, stop=True,
                )
                nc.vector.tensor_reduce(
                    out=mbuf[:, t:t + 1], in_=dist, axis=AX, op=OP.min
                )
                sgn.append((dist, wfg, j, t))
                # pair-batched bias: b' = m/delta + 1
                if len(sgn) == 2 or t == tiles - 1:
                    q0 = sgn[0][3]
                    nc.vector.tensor_scalar(
                        out=bbuf[:, q0:t + 1], in0=mbuf[:, q0:t + 1],
                        scalar1=1.0 / DELTA, scalar2=1.0,
                        op0=OP.mult, op1=OP.add,
                    )
                    for sd, sw, sj, st in sgn:
                        a_t = ap_.tile([P, K], bf16, tag="a_t")
                        nc.scalar.activation(
                            out=a_t, in_=sd, func=AF.Relu,
                            bias=bbuf[:, st:st + 1], scale=-1.0 / DELTA,
                        )
                        pend.append((a_t, sw, sj, st))
                    sgn = []
                while len(pend) > 4:
                    pa, pw, pj, pt = pend.pop(0)
                    nc.tensor.matmul(
                        scat, pw[:, pj, :], pa,
                        start=(pt == 0), stop=(pt == tiles - 1),
                    )
                t += 1
            base_row += g * P
        for pa, pw, pj, pt in pend:
            nc.tensor.matmul(
                scat, pw[:, pj, :], pa,
                start=(pt == 0), stop=(pt == tiles - 1),
            )

        # ---- finalize ----
        out_sb = consts.tile([P, K + 4], fp32)
        nc.vector.tensor_copy(out_sb[:, :K], scat)
        # inertia partials: sum sqrt(clip(m+xx+cc0,0)/64); cc0 re-adds the
        # host-side centering of the CC row (argmin is shift-invariant)
        tt_b = consts.tile([P, tiles], fp32)
        nc.vector.tensor_tensor(out=tt_b, in0=mbuf, in1=xxbuf, op=OP.add)
        nc.vector.tensor_scalar(
            out=tt_b, in0=tt_b, scalar1=cc0_sb, scalar2=0.0,
            op0=OP.add, op1=OP.max,
        )
        sq_b = consts.tile([P, tiles], fp32)
        inert = consts.tile([P, 1], fp32)
        nc.scalar.activation(
            out=sq_b, in_=tt_b, func=AF.Sqrt, scale=1.0 / D, accum_out=inert
        )
        nc.vector.tensor_copy(out_sb[:, K:K + 1], inert)
        nc.sync.dma_start(out=out.ap(), in_=out_sb)

    _split_multi_waits(nc, mybir)
    return nc


def _split_multi_waits(nc, mybir):
    """This walrus build allows max 1 sem-wait per instruction: hoist extras
    onto inserted NoOps on the same engine queue."""
    import copy

    module = nc.m
    new_module = copy.replace(module, functions=[])
    for function in module.functions:
        new_function = copy.replace(function, blocks=[])
        new_function.set_allocations_from_list(function.allocations)
        for block in function.blocks:
            new_insts = []
            for ins in block.instructions:
                si = ins.sync_info
                if si is not None and si.on_wait and len(si.on_wait) > 1:
                    waits = list(si.on_wait)
                    for k, w in enumerate(waits[:-1]):
                        new_insts.append(mybir.InstNoOp(
                            name=f"{ins.name}-wsplit{k}", engine=ins.engine,
                            ins=[], outs=[],
                            sync_info=mybir.SyncInfo(on_wait=[w], on_update=[]),
                        ))
                    ins.sync_info = mybir.SyncInfo(
                        on_wait=[waits[-1]], on_update=list(si.on_update or [])
                    )
                new_insts.append(ins)
            new_function.blocks.append(copy.replace(block, instructions=new_insts))
        new_module.functions.append(new_function)
    nc.m = new_module


def _prep_inputs(X, centroids, sample_weight):
    C = np.asarray(centroids, dtype=np.float32)
    X = np.asarray(X, dtype=np.float32)
    W = np.asarray(sample_weight, dtype=np.float32)
    Xp = np.empty((NPAD, D), dtype=np.float32)
    Xp[:N] = X
    Xp[N:] = C[0]
    Wp = np.zeros((NPAD, D), dtype=np.float32)
    Wp[:N] = W
    CC = (C ** 2).sum(axis=1)
    cc0 = np.float32(CC.mean())
    CCc = CC - cc0  # centered: argmin is shift-invariant
    cc_hi = CCc.astype(ml_dtypes.bfloat16)
    cc_lo = (CCc - cc_hi.astype(np.float32)).astype(ml_dtypes.bfloat16)
    rhs = np.empty((D + 2, K), dtype=ml_dtypes.bfloat16)
    rhs[:D] = (-2.0 * C.T).astype(ml_dtypes.bfloat16)
    rhs[D] = cc_hi
    rhs[D + 1] = cc_lo
    cc0_t = np.full((P, 1), cc0, dtype=np.float32)
    XXp = (Xp * Xp).sum(axis=1)
    idx = _CACHE.setdefault("idx", _row_index())
    ones = np.ones((2, ROWS), dtype=ml_dtypes.bfloat16)
    in_maps = []
    for c in range(NCORES):
        sl = slice(c * ROWS, (c + 1) * ROWS)
        Xc, Wc = Xp[sl], Wp[sl]
        XT = np.concatenate(
            [Xc[idx].T.astype(ml_dtypes.bfloat16), ones], axis=0
        )
        WF = np.concatenate([Xc * Wc, Wc], axis=1).astype(ml_dtypes.bfloat16)
        XXc = np.ascontiguousarray(XXp[sl][idx].reshape(TILES, P).T)
        in_maps.append({"wf": WF, "xt": np.ascontiguousarray(XT), "rhs": rhs,
                        "xx": XXc, "cc0": cc0_t})
    return in_maps


def run(X, centroids, sample_weight, trace=False):
    from concourse.bass_utils import run_bass_kernel_spmd

    if "nc" not in _CACHE:
        _CACHE["nc"] = _build()
    in_maps = _prep_inputs(X, centroids, sample_weight)
    res = run_bass_kernel_spmd(
        _CACHE["nc"], in_maps, core_ids=list(range(NCORES)), trace=trace
    )
    xw = np.zeros((K, D), dtype=np.float64)
    ws = np.zeros((K, D), dtype=np.float64)
    inertia = 0.0
    for c in range(NCORES):
        o = res.results[c]["out"]
        xw += o[:D, :K].T.astype(np.float64)
        ws += o[D:2 * D, :K].T.astype(np.float64)
        inertia += float(o[:, K].sum(dtype=np.float64))
    packed = np.concatenate(
        [xw, ws, np.full((1, D), inertia)], axis=0
    ).astype(np.float32)
    return packed, res


def kernel(X, centroids, sample_weight):
    packed, _ = run(X, centroids, sample_weight)
    return packed


# revision 12
# speedup vs baseline: 1.1957x; 1.1957x over previous
"""KMeans summarize kernel for Trainium2, data-parallel over N on 8 NeuronCores.

v3: all-bf16 matmuls (FWL-fast weight loads), Relu one-hot in {0,1}.

Per core (62592 rows = 489 tiles of 128):
  PE:   q = CC' - 2*X@C^T via bf16 matmul (stationary [X^T;1;1], moving
        [-2C^T; CChi; CClo], fp32 PSUM accumulate); scatter
        acc[j,k] += sum_i M[i,j]*A[i,k] with M=[X*W | W] bf16, A one-hot
  DVE:  m = min_k q per row; b' = m/delta + 1 (quad-batched)
  ACT:  A = Relu(-q/delta + b')  -- exactly 1 at the argmin, 0 elsewhere
        (Sterbenz: at argmin q==m so -m/d + m/d + 1 == 1 exactly)
Host: XW=X*W and XX=sum X^2 precomputed; CC row centered and split into
bf16 hi/lo pair. inertia = sum sqrt(clip(XX+m+cc0,0)/64) on device.
"""

import sys

sys.path.insert(0, "/opt/trn_rl_repo")

import numpy as np
import ml_dtypes

N, D, K = 500_000, 64, 512
NCORES = 8
P = 128
TILES = 489                      # per-core tiles
ROWS = P * TILES                 # 62592 per core
NPAD = NCORES * ROWS             # 500736
GROUP = 8
DELTA = 2.0 ** -12
# First group split small so the first matmul's DMA lands early.
_GROUPS = [2, 2, 4] + [GROUP] * (TILES // GROUP - 1) + (
    [TILES % GROUP] if TILES % GROUP else [])

_CACHE = {}


def _row_index(tiles=TILES, groups=_GROUPS):
    """Original row index for (tile t, partition p) ordering used on device."""
    idx = np.empty(tiles * P, dtype=np.int64)
    base = 0
    t = 0
    for g in groups:
        for j in range(g):
            tt = t + j
            idx[tt * P:(tt + 1) * P] = base + np.arange(P) * g + j
        base += g * P
        t += g
    return idx


def _build(groups=None):
    import concourse.bass as bass
    import concourse.mybir as mybir
    import concourse.tile as tile

    if groups is None:
        groups = _GROUPS
    tiles = sum(groups)

    fp32 = mybir.dt.float32
    bf16 = mybir.dt.bfloat16
    nc = bass.Bass()
    wf = nc.dram_tensor("wf", (ROWS, 2 * D), bf16, kind="ExternalInput")
    xt = nc.dram_tensor("xt", (D + 2, ROWS), bf16, kind="ExternalInput")
    rhs = nc.dram_tensor("rhs", (D + 2, K), bf16, kind="ExternalInput")
    xx = nc.dram_tensor("xx", (P, tiles), fp32, kind="ExternalInput")
    cc0 = nc.dram_tensor("cc0", (P, 1), fp32, kind="ExternalInput")
    out = nc.dram_tensor("out", (P, K + 4), fp32, kind="ExternalOutput")

    AX = mybir.AxisListType.X
    OP = mybir.AluOpType
    AF = mybir.ActivationFunctionType

    from contextlib import ExitStack

    with tile.TileContext(nc) as tc, ExitStack() as es:
        consts = es.enter_context(tc.tile_pool(name="consts", bufs=1))
        xtp = es.enter_context(tc.tile_pool(name="xtp", bufs=3))
        wfp = es.enter_context(tc.tile_pool(name="wfp", bufs=3))
        ap_ = es.enter_context(tc.tile_pool(name="ap_", bufs=7))
        dpp = es.enter_context(tc.tile_pool(name="dpp", bufs=6, space="PSUM"))
        app = es.enter_context(tc.tile_pool(name="app", bufs=1, space="PSUM"))

        rhs_sb = consts.tile([D + 2, K], bf16)
        nc.sync.dma_start(out=rhs_sb, in_=rhs.ap())
        xxbuf = consts.tile([P, tiles], fp32)
        nc.sync.dma_start(out=xxbuf, in_=xx.ap())
        cc0_sb = consts.tile([P, 1], fp32)
        nc.sync.dma_start(out=cc0_sb, in_=cc0.ap())
        mbuf = consts.tile([P, tiles], fp32)
        bbuf = consts.tile([P, tiles], fp32)
        scat = app.tile([P, K], fp32)            # scatter accumulator (1 bank)

        pend = []          # (a_tile, wfg, j, t) awaiting scatter matmul
        sgn = []           # (dist, wfg, j, t) awaiting bias + relu
        t = 0
        base_row = 0
        for g in groups:
            # ---- group loads ----
            xtg = xtp.tile([D + 2, GROUP * P], bf16, tag="xtg")
            nc.sync.dma_start(
                out=xtg[:, : g * P], in_=xt.ap()[:, t * P:(t + g) * P]
            )
            wfg = wfp.tile([P, GROUP, 2 * D], bf16, tag="wfg")
            src_w = bass.AP(
                tensor=wf, offset=base_row * 2 * D,
                ap=[[g * 2 * D, P], [2 * D, g], [1, 2 * D]],
            )
            nc.sync.dma_start(out=wfg[:, :g, :], in_=src_w)

            for j in range(g):
                dist = dpp.tile([P, K], fp32, tag="dist")
                nc.tensor.matmul(
                    dist, xtg[:, j * P:(j + 1) * P], rhs_sb,
                    start=True, stop=True,
                )
                nc.vector.tensor_reduce(
                    out=mbuf[:, t:t + 1], in_=dist, axis=AX, op=OP.min
                )
                sgn.append((dist, wfg, j, t))
                # pair-batched bias: b' = m/delta + 1
                if len(sgn) == 2 or t == tiles - 1:
                    q0 = sgn[0][3]
                    nc.gpsimd.tensor_scalar(
                        out=bbuf[:, q0:t + 1], in0=mbuf[:, q0:t + 1],
                        scalar1=1.0 / DELTA, scalar2=1.0,
                        op0=OP.mult, op1=OP.add,
                    )
                    for sd, sw, sj, st in sgn:
                        a_t = ap_.tile([P, K], bf16, tag="a_t")
                        nc.scalar.activation(
                            out=a_t, in_=sd, func=AF.Relu,
                            bias=bbuf[:, st:st + 1], scale=-1.0 / DELTA,
                        )
                        pend.append((a_t, sw, sj, st))
                    sgn = []
                while len(pend) > 3:
                    pa, pw, pj, pt = pend.pop(0)
                    nc.tensor.matmul(
                        scat, pw[:, pj, :], pa,
                        start=(pt == 0), stop=(pt == tiles - 1),
                    )
                t += 1
            base_row += g * P
        for pa, pw, pj, pt in pend:
            nc.tensor.matmul(
                scat, pw[:, pj, :], pa,
                start=(pt == 0), stop=(pt == tiles - 1),
            )

        # ---- finalize ----
        out_sb = consts.tile([P, K + 4], fp32)
        nc.vector.tensor_copy(out_sb[:, :K], scat)
        # inertia partials: sum sqrt(clip(m+xx+cc0,0)/64); cc0 re-adds the
        # host-side centering of the CC row (argmin is shift-invariant)
        tt_b = consts.tile([P, tiles], fp32)
        nc.vector.tensor_tensor(out=tt_b, in0=mbuf, in1=xxbuf, op=OP.add)
        nc.vector.tensor_scalar(
            out=tt_b, in0=tt_b, scalar1=cc0_sb, scalar2=0.0,
            op0=OP.add, op1=OP.max,
        )
        sq_b = consts.tile([P, tiles], fp32)
        inert = consts.tile([P, 1], fp32)
        nc.scalar.activation(
            out=sq_b, in_=tt_b, func=AF.Sqrt, scale=1.0 / D, accum_out=inert
        )
        nc.vector.tensor_copy(out_sb[:, K:K + 1], inert)
        nc.sync.dma_start(out=out.ap(), in_=out_sb)

    _split_multi_waits(nc, mybir)
    return nc


def _split_multi_waits(nc, mybir):
    """This walrus build allows max 1 sem-wait per instruction: hoist extras
    onto inserted NoOps on the same engine queue."""
    import copy

    module = nc.m
    new_module = copy.replace(module, functions=[])
    for function in module.functions:
        new_function = copy.replace(function, blocks=[])
        new_function.set_allocations_from_list(function.allocations)
        for block in function.blocks:
            new_insts = []
            for ins in block.instructions:
                si = ins.sync_info
                if si is not None and si.on_wait and len(si.on_wait) > 1:
                    waits = list(si.on_wait)
                    for k, w in enumerate(waits[:-1]):
                        new_insts.append(mybir.InstNoOp(
                            name=f"{ins.name}-wsplit{k}", engine=ins.engine,
                            ins=[], outs=[],
                            sync_info=mybir.SyncInfo(on_wait=[w], on_update=[]),
                        ))
                    ins.sync_info = mybir.SyncInfo(
                        on_wait=[waits[-1]], on_update=list(si.on_update or [])
                    )
                new_insts.append(ins)
            new_function.blocks.append(copy.replace(block, instructions=new_insts))
        new_module.functions.append(new_function)
    nc.m = new_module


def _prep_inputs(X, centroids, sample_weight):
    C = np.asarray(centroids, dtype=np.float32)
    X = np.asarray(X, dtype=np.float32)
    W = np.asarray(sample_weight, dtype=np.float32)
    Xp = np.empty((NPAD, D), dtype=np.float32)
    Xp[:N] = X
    Xp[N:] = C[0]
    Wp = np.zeros((NPAD, D), dtype=np.float32)
    Wp[:N] = W
    CC = (C ** 2).sum(axis=1)
    cc0 = np.float32(CC.mean())
    CCc = CC - cc0  # centered: argmin is shift-invariant
    cc_hi = CCc.astype(ml_dtypes.bfloat16)
    cc_lo = (CCc - cc_hi.astype(np.float32)).astype(ml_dtypes.bfloat16)
    rhs = np.empty((D + 2, K), dtype=ml_dtypes.bfloat16)
    rhs[:D] = (-2.0 * C.T).astype(ml_dtypes.bfloat16)
    rhs[D] = cc_hi
    rhs[D + 1] = cc_lo
    cc0_t = np.full((P, 1), cc0, dtype=np.float32)
    XXp = (Xp * Xp).sum(axis=1)
    idx = _CACHE.setdefault("idx", _row_index())
    ones = np.ones((2, ROWS), dtype=ml_dtypes.bfloat16)
    in_maps = []
    for c in range(NCORES):
        sl = slice(c * ROWS, (c + 1) * ROWS)
        Xc, Wc = Xp[sl], Wp[sl]
        XT = np.concatenate(
            [Xc[idx].T.astype(ml_dtypes.bfloat16), ones], axis=0
        )
        WF = np.concatenate([Xc * Wc, Wc], axis=1).astype(ml_dtypes.bfloat16)
        XXc = np.ascontiguousarray(XXp[sl][idx].reshape(TILES, P).T)
        in_maps.append({"wf": WF, "xt": np.ascontiguousarray(XT), "rhs": rhs,
                        "xx": XXc, "cc0": cc0_t})
    return in_maps


def run(X, centroids, sample_weight, trace=False):
    from concourse.bass_utils import run_bass_kernel_spmd

    if "nc" not in _CACHE:
        _CACHE["nc"] = _build()
    in_maps = _prep_inputs(X, centroids, sample_weight)
    res = run_bass_kernel_spmd(
        _CACHE["nc"], in_maps, core_ids=list(range(NCORES)), trace=trace
    )
    xw = np.zeros((K, D), dtype=np.float64)
    ws = np.zeros((K, D), dtype=np.float64)
    inertia = 0.0
    for c in range(NCORES):
        o = res.results[c]["out"]
        xw += o[:D, :K].T.astype(np.float64)
        ws += o[D:2 * D, :K].T.astype(np.float64)
        inertia += float(o[:, K].sum(dtype=np.float64))
    packed = np.concatenate(
        [xw, ws, np.full((1, D), inertia)], axis=0
    ).astype(np.float32)
    return packed, res


def kernel(X, centroids, sample_weight):
    packed, _ = run(X, centroids, sample_weight)
    return packed
